# revision 14
# baseline (speedup 1.0000x reference)
"""Trainium2 Bass kernel for nn_MessagePassingLayer (graph U-Net message
passing) on 8 NeuronCores.

Self-contained: kernel(**inputs) takes the full unsharded inputs and
returns the full [50000, 128] float32 output.

Strategy: nodes padded to 50176 and sharded contiguously over the 8
cores; edges bucketed per (dst block, src half) in destination-sorted
order. Every per-edge coefficient factorizes as rowfac[row] * colfac[col],
so row factors are pre-applied to the gathered node table (bf16 XS
buffers, exchanged via AllGather) and col factors are applied on PSUM
eviction. Edge pipeline runs in bf16: dma_gather of 256B rows, one-hot
tiles in transposed [128, dst, tile] layout (DVE 2x mode) + PE matmul
accumulation per 128-dst block, then W matmul + rank-1 bias + PE
transpose. Top-k pooling is an on-device replicated fp32 threshold
bisection; degree/weight renormalization uses narrow bf16 Z-table
gather passes.
"""
import math

import numpy as np
from ml_dtypes import bfloat16

import concourse.bacc as bacc
import concourse.mybir as mybir
import concourse.tile as tile
from concourse.bass_utils import run_bass_kernel_spmd

from dataclasses import dataclass, field

P = 128
NCORES = 8
GROUP_BLOCKS = 5


@dataclass
class Cfg:
    N: int = 50000
    E: int = 800000
    D: int = 128
    L: int = 2
    B: int = 2
    ratio: float = 0.5
    blocks_per_core: int = field(init=False)
    N_pad: int = field(init=False)
    nodes_per_core: int = field(init=False)
    half: int = field(init=False)          # rows per gather half-table

    def __post_init__(self):
        blocks_total = math.ceil(self.N / P)
        self.blocks_per_core = math.ceil(blocks_total / NCORES)
        self.N_pad = self.blocks_per_core * NCORES * P
        self.nodes_per_core = self.blocks_per_core * P
        self.half = self.N_pad // 2
        assert self.half <= 32768, "gather half-table must fit int16 index"
        assert self.half % P == 0


def wrap_idx(idx, n):
    """[n] int -> [128, n/16] int16 wrapped+replicated layout for dma_gather."""
    assert n % 16 == 0
    w = np.zeros((16, n // 16), np.int16)
    w[np.arange(n) % 16, np.arange(n) // 16] = idx.astype(np.int16)
    return np.tile(w, (8, 1))


def build_direction(cfg, src, dst, group_blocks):
    """Static tables for one scatter direction (gather at src, scatter to dst).

    Edges bucketed per (core, local dst block, src half), padded to tiles
    of 128 with null edges. Tile counts per (block, half) are maxed across
    cores so the 8 cores share one instruction stream; per (group, half)
    tile sums are padded to even so one-hot tiles stay 4B-aligned in bf16.
    """
    bpc = cfg.blocks_per_core
    npc = cfg.nodes_per_core
    half = cfg.half
    zero_row = cfg.N_pad - 1          # pad node: always-zero row (half 1)

    buckets = [[[None, None] for _ in range(bpc)] for _ in range(NCORES)]
    core_of = dst // npc
    blk_of = (dst % npc) // P
    half_of = (src >= half).astype(np.int64)
    order = np.lexsort((src, half_of, blk_of, core_of))
    key = ((core_of[order] * bpc) + blk_of[order]) * 2 + half_of[order]
    bounds = np.searchsorted(key, np.arange(NCORES * bpc * 2 + 1))
    for c in range(NCORES):
        for b in range(bpc):
            for h in (0, 1):
                kk = (c * bpc + b) * 2 + h
                buckets[c][b][h] = order[bounds[kk]:bounds[kk + 1]]

    T = np.zeros((bpc, 2), np.int64)
    for b in range(bpc):
        for h in (0, 1):
            mx = max(len(buckets[c][b][h]) for c in range(NCORES))
            T[b, h] = max(1, math.ceil(mx / P))

    groups = []
    for g0 in range(0, bpc, group_blocks):
        groups.append(list(range(g0, min(g0 + group_blocks, bpc))))

    # pad per-(group, half) tile counts to even (bump last block in group)
    for grp in groups:
        for h in (0, 1):
            if sum(int(T[b, h]) for b in grp) % 2:
                T[grp[-1], h] += 1

    tot_tiles = int(T.sum())
    E_flat = tot_tiles * P
    idxs = np.zeros((NCORES, E_flat), np.int64)
    dstl = np.full((NCORES, E_flat), 200.0, np.float32)
    pos = 0
    tile_plan = []   # (h, b, ntiles, start_pos) shared across cores
    for grp in groups:
        for h in (0, 1):
            for b in grp:
                nt = int(T[b, h])
                tile_plan.append((h, b, nt, pos))
                for c in range(NCORES):
                    ed = buckets[c][b][h]
                    n = len(ed)
                    assert n <= nt * P
                    idxs[c, pos:pos + n] = src[ed] - h * half
                    dstl[c, pos:pos + n] = (dst[ed] % npc) % P
                    # null edges: h=1 gathers the pad zero row; h=0 gathers
                    # real row 0 but dstl=200 never matches the iota
                    if n < nt * P:
                        idxs[c, pos + n:pos + nt * P] = (zero_row - half) if h else 0
                pos += nt * P
    assert pos == E_flat

    idx16 = np.stack([wrap_idx(idxs[c], E_flat) for c in range(NCORES)])
    ntiles = E_flat // P
    dstl_t = dstl.reshape(NCORES, ntiles, P).transpose(0, 2, 1).astype(bfloat16)

    return {
        "idx16": idx16,            # [NCORES, 128, E_flat/16] int16
        "dstl": dstl_t,            # [NCORES, 128, ntiles] bf16
        "tile_plan": tile_plan,
        "groups": groups,
        "E_flat": E_flat,
        "ntiles": ntiles,
        "T": T,
    }


def preprocess(cfg, x, edge_index, pvec):
    """All static host work. Returns per-core input pieces + meta."""
    N, Np = cfg.N, cfg.N_pad
    row = edge_index[0].astype(np.int64)
    col = edge_index[1].astype(np.int64)

    deg0 = np.zeros(Np, np.float32)
    np.add.at(deg0, row, 1.0)
    with np.errstate(divide="ignore"):
        dis0 = np.where(deg0 > 0, deg0.astype(np.float64) ** -0.5, 0.0).astype(np.float32)
    normed0 = np.where(deg0 > 0, 1.0 / np.where(deg0 > 0, deg0, 1), 0.0).astype(np.float32)
    A0 = np.zeros(Np, np.float32)
    np.add.at(A0, col, normed0[row])
    aggr0 = (A0 + 1e-12).astype(np.float32)
    r0 = (1.0 / aggr0).astype(np.float32)
    q0 = normed0
    u0 = dis0
    sigma0 = np.zeros(Np, np.float32)
    np.add.at(sigma0, col, u0[row])

    cs = build_direction(cfg, row, col, GROUP_BLOCKS)
    rs = build_direction(cfg, col, row, GROUP_BLOCKS)

    xs0 = np.zeros((Np, cfg.D), np.float32)
    xs0[:N] = x * u0[:N, None]

    realmask = np.zeros(Np, np.float32)
    realmask[:N] = 1.0

    p0 = pvec[0] / np.linalg.norm(pvec[0])
    p1 = pvec[1] / np.linalg.norm(pvec[1])

    def shardvec(v):   # [Np] -> [NCORES, 128, bpc]
        return v.reshape(NCORES, cfg.blocks_per_core, P).transpose(0, 2, 1).copy()

    def fullvec(v):    # [Np] -> [128, blocks_total]
        return v.reshape(-1, P).T.copy()

    return {
        "cs": cs, "rs": rs,
        "u0": u0, "q0": q0, "r0": r0, "aggr0": aggr0, "sigma0": sigma0,
        "xs0": xs0, "realmask": realmask, "p0": p0, "p1": p1,
        "shardvec": shardvec, "fullvec": fullvec,
    }


F32 = mybir.dt.float32
BF16 = mybir.dt.bfloat16
I16 = mybir.dt.int16
AF = mybir.ActivationFunctionType
OP = mybir.AluOpType
AX = mybir.AxisListType

ZCHUNK = 40
ZW = 128          # bf16 z rows must be 256B for dma_gather
BISECT_ITERS = 40


class G:
    """build-time globals bag"""
    pass


# ------------------------------------------------------------- edge passes --

def emit_dir_pass(g, dirn, src_dram, mode, block_fn, outs=(), zcols=0):
    """One edge pass.

    mode: 'conv' / 'wec' (psum [feat, dst], block_fn returns final psum
    [dst, feat] for eviction) or 'z' (psum [zcols, dst], no eviction).
    outs: eviction specs applied per group with batched DMA.
    """
    nc = g.nc
    d = g.dirs[dirn]
    elem = ZW if mode == "z" else g.D
    if mode == "z":
        in_aps = [g.z_dram.ap()[0:g.half, :], g.z_dram.ap()[g.half:, :]]
    else:
        in_aps = [src_dram.ap()[0:g.half, :], src_dram.ap()[g.half:, :]]

    gtag = "fg"
    plan = {(h, b): (nt, pos) for (h, b, nt, pos) in d["tile_plan"]}
    NB = GROUP_BLOCKS
    for grp in d["groups"]:
        t0g = plan[(0, grp[0])][1] // P
        ntg_h = [sum(plan[(h, b)][0] for b in grp) for h in (0, 1)]
        ntg_all = ntg_h[0] + ntg_h[1]
        it = g.sb_idx.tile([128, 2 * g.maxtg * 8], I16, name="idx", tag="idx")
        nc.sync.dma_start(out=it[:, :ntg_all * 8],
                          in_=d["idx16_d"].ap()[:, t0g * 8: t0g * 8 + ntg_all * 8])
        dlg = g.sb_idx.tile([128, 2 * g.maxtg], BF16, name="dl", tag="dl")
        nc.sync.dma_start(out=dlg[:, :ntg_all],
                          in_=d["dstl_d"].ap()[:, t0g: t0g + ntg_all])
        parts = {}
        for h in (0, 1):
            off = 0 if h == 0 else ntg_h[0]
            ntg = ntg_h[h]
            gt = g.sb_gath.tile([P, g.maxtg, elem], BF16, name="fg", tag=gtag)
            nc.gpsimd.dma_gather(
                out_ap=gt[:, :ntg, :], in_ap=in_aps[h],
                idxs_ap=it[:, off * 8: off * 8 + ntg * 8],
                num_idxs=ntg * P, num_idxs_reg=ntg * P, elem_size=elem,
                single_packet=False)
            oh = g.sb_oht.tile([P, P, g.maxtg], BF16, name="oht", tag="oht")
            nc.vector.tensor_tensor(
                out=oh[:, :, :ntg],
                in0=dlg[:, None, off: off + ntg].to_broadcast([P, P, ntg]),
                in1=g.iota_m[:, :, :ntg], op=OP.is_equal)
            parts[h] = (gt, oh)
        gbufs = [g.sb_out.tile([P, NB, P], o[1], name="gb", tag=f"gb{o[1]}{i}")
                 for i, o in enumerate(outs)]
        # per-block interleaved halves so only a couple agg psums are live
        cum = {0: 0, 1: 0}
        for j, b in enumerate(grp):
            psum = g.ps_agg.tile([P, P], F32, space="PSUM", name="agg", tag="agg")
            tot = plan[(0, b)][0] + plan[(1, b)][0]
            k = 0
            for h in (0, 1):
                gt, oh = parts[h]
                nt, _pos = plan[(h, b)]
                tpos = cum[h]
                for t in range(nt):
                    k += 1
                    if mode == "z":
                        lhs = gt[:, tpos + t, :zcols]
                        outp = psum[:zcols, :]
                    else:
                        lhs = gt[:, tpos + t, :]
                        outp = psum[:]
                    nc.tensor.matmul(out=outp, lhsT=lhs, rhs=oh[:, :, tpos + t],
                                     start=(k == 1), stop=(k == tot))
                cum[h] += nt
            fin = block_fn(b, psum[:zcols, :] if mode == "z" else psum)
            if fin is not None:
                for gb, o in zip(gbufs, outs):
                    nc.scalar.activation(
                        out=gb[:, j, :], in_=fin[:], func=AF.Copy,
                        scale=o[2][:, b: b + 1])
        # flush group evictions
        b0 = grp[0]
        nb = len(grp)
        for gb, o in zip(gbufs, outs):
            kind, dt_ = o[0], o[1]
            dst_dram = o[3]
            dview = dst_dram.ap().rearrange("(b p) d -> p b d", p=P)[:, b0: b0 + nb]
            if kind == "xs":
                nc.sync.dma_start(out=dview, in_=gb[:, :nb, :])
            else:  # add: (kind, dtype, pre, out_dram, in_dram, post)
                in_dram, post = o[4], o[5]
                sk = g.sb_out.tile([P, NB, P], BF16, name="skl", tag="skl")
                nc.sync.dma_start(
                    out=sk[:, :nb, :],
                    in_=in_dram.ap().rearrange("(b p) d -> p b d", p=P)[:, b0: b0 + nb])
                nc.vector.tensor_tensor(out=gb[:, :nb, :], in0=gb[:, :nb, :],
                                        in1=sk[:, :nb, :], op=OP.add)
                if post is not None:
                    for j, b in enumerate(grp):
                        nc.vector.tensor_scalar(
                            out=gb[:, j, :], in0=gb[:, j, :],
                            scalar1=post[:, b: b + 1], scalar2=None, op0=OP.mult)
                nc.sync.dma_start(out=dview, in_=gb[:, :nb, :])


def conv_block_fn(g, W_sb, b_sb, sigma_row):
    nc = g.nc

    def fn(b, pag):
        a1 = g.sb_ev.tile([P, P], BF16, name="a1", tag="a1")
        nc.vector.tensor_copy(out=a1[:], in_=pag[:])
        p2 = g.ps_w.tile([P, P], F32, space="PSUM", name="p2", tag="p2")
        nc.tensor.matmul(out=p2[:], lhsT=W_sb[:], rhs=a1[:], start=True, stop=False)
        nc.tensor.matmul(out=p2[:], lhsT=b_sb[:],
                         rhs=sigma_row[:, b * P:(b + 1) * P],
                         start=False, stop=True)
        a2 = g.sb_ev.tile([P, P], BF16, name="a2", tag="a2")
        nc.vector.tensor_copy(out=a2[:], in_=p2[:])
        p3 = g.ps_t.tile([P, P], BF16, space="PSUM", name="pst", tag="pst")
        nc.tensor.transpose(out=p3[:], in_=a2[:], identity=g.identb[:])
        return p3
    return fn


def wec_block_fn(g, score_to=None, colfac=None, pcol=None):
    nc = g.nc

    def fn(b, pag):
        a1 = g.sb_ev.tile([P, P], BF16, name="a1", tag="a1")
        nc.vector.tensor_copy(out=a1[:], in_=pag[:])
        if score_to is not None:
            psc = g.ps_w.tile([P, P], F32, space="PSUM", name="psc", tag="p2")
            nc.tensor.matmul(out=psc[:, :1], lhsT=a1[:], rhs=pcol[:],
                             start=True, stop=True)
            nc.vector.tensor_tensor(out=score_to[:, b: b + 1], in0=psc[:, :1],
                                    in1=colfac[:, b: b + 1], op=OP.mult)
        p3 = g.ps_t.tile([P, P], BF16, space="PSUM", name="pst", tag="pst")
        nc.tensor.transpose(out=p3[:], in_=a1[:], identity=g.identb[:])
        return p3
    return fn


def z_block_fn(g, row_to, col_to, zcols):
    """row_to: [(rowtile, j)] copy psum row j (bf16 cast); col_to: fp32 cols."""
    nc = g.nc

    def fn(b, pag):
        for (rt, j) in row_to:
            nc.vector.tensor_copy(out=rt[:, b * P:(b + 1) * P], in_=pag[j:j + 1, :])
        if col_to:
            az = g.sb_ev.tile([P, P], F32, name="az", tag="az")
            nc.vector.tensor_copy(out=az[:zcols, :], in_=pag[:])
            pz = g.ps_w.tile([P, P], F32, space="PSUM", name="pz", tag="p2")
            nc.tensor.transpose(out=pz[:, :zcols], in_=az[:zcols, :],
                                identity=g.ident[:zcols, :zcols])
            for (ct, j) in col_to:
                nc.vector.tensor_copy(out=ct[:, b: b + 1], in_=pz[:, j: j + 1])
        return None
    return fn


# ------------------------------------------------------------- small pieces --

def allgather(g, in_dram, out_dram):
    g.nc.gpsimd.collective_compute(
        "AllGather", OP.bypass, replica_groups=[list(range(NCORES))],
        ins=[in_dram.ap()], outs=[out_dram.ap()])


def zbuild(g, cols):
    nc = g.nc
    for c0 in range(0, g.BT, ZCHUNK):
        nb = min(ZCHUNK, g.BT - c0)
        st = g.sb_zst.tile([P, ZCHUNK, ZW], BF16, name="zst", tag="zst")
        for j, v in enumerate(cols):
            nc.vector.tensor_copy(out=st[:, :nb, j:j + 1], in_=v[:, c0:c0 + nb, None])
        nc.sync.dma_start(
            out=g.z_dram.ap().rearrange("(b p) w -> p b w", p=P)[:, c0:c0 + nb, :],
            in_=st[:, :nb, :])


def cross_part(g, col, op):
    nc = g.nc
    if op == "sum":
        pc = g.ps_w.tile([P, P], F32, space="PSUM", name="pc", tag="p2")
        nc.tensor.matmul(out=pc[:1, :1], lhsT=col[:], rhs=g.ones_col[:],
                         start=True, stop=True)
        out = g.sb_bis.tile([1, 1], F32, name="cnt", tag="cnt")
        nc.vector.tensor_copy(out=out[:], in_=pc[:1, :1])
        return out
    pt = g.ps_w.tile([P, P], F32, space="PSUM", name="pt", tag="p2")
    nc.tensor.transpose(out=pt[:1, :], in_=col[:], identity=g.ident[:])
    row = g.sb_bis.tile([1, P], F32, name="brow", tag="brow")
    nc.vector.tensor_copy(out=row[:], in_=pt[:1, :])
    out = g.sb_bis.tile([1, 1], F32, name="bred", tag="bred")
    nc.vector.reduce_max(out[:], row[:], axis=AX.X)
    return out


def bcast_scalar(g, s11, tag):
    nc = g.nc
    pb = g.ps_w.tile([P, P], F32, space="PSUM", name="pb", tag="p2")
    nc.tensor.matmul(out=pb[:, :1], lhsT=g.ones_row[:], rhs=s11[:],
                     start=True, stop=True)
    out = g.sb_nv.tile([P, 1], F32, name=tag, tag=tag)
    nc.vector.tensor_copy(out=out[:], in_=pb[:, :1])
    return out


def bisect_topk(g, sel_full, k, tag):
    """threshold col [128,1] such that count(sel >= t) == k exactly."""
    nc = g.nc
    mx = g.sb_bis.tile([P, 1], F32, name="bmx", tag="bmx")
    nc.vector.reduce_max(mx[:], sel_full[:], axis=AX.X)
    hi = cross_part(g, mx, "max")
    nc.vector.tensor_scalar(out=hi[:], in0=hi[:], scalar1=1.0, scalar2=None,
                            op0=OP.add)
    lo = g.sb_bis.tile([1, 1], F32, name="blo", tag="blo")
    nc.vector.tensor_scalar(out=lo[:], in0=hi[:], scalar1=-4e4, scalar2=None,
                            op0=OP.add)
    t = g.sb_bis.tile([1, 1], F32, name="bt", tag="bt")
    for _ in range(BISECT_ITERS):
        nc.vector.tensor_tensor(out=t[:], in0=lo[:], in1=hi[:], op=OP.add)
        nc.vector.tensor_scalar(out=t[:], in0=t[:], scalar1=0.5, scalar2=None,
                                op0=OP.mult)
        tcol = bcast_scalar(g, t, "btc")
        cmp = g.sb_bis.tile([P, g.BT], F32, name="bcmp", tag="bcmp")
        nc.vector.tensor_scalar(out=cmp[:], in0=sel_full[:], scalar1=tcol[:],
                                scalar2=None, op0=OP.is_ge)
        red = g.sb_bis.tile([P, 1], F32, name="bred2", tag="bred2")
        nc.vector.reduce_sum(red[:], cmp[:], axis=AX.X)
        cnt = cross_part(g, red, "sum")
        flag = g.sb_bis.tile([1, 1], F32, name="bflag", tag="bflag")
        nc.vector.tensor_scalar(out=flag[:], in0=cnt[:], scalar1=float(k) - 0.5,
                                scalar2=None, op0=OP.is_ge)
        d1 = g.sb_bis.tile([1, 1], F32, name="bd1", tag="bd1")
        nc.vector.tensor_tensor(out=d1[:], in0=t[:], in1=lo[:], op=OP.subtract)
        nc.vector.tensor_tensor(out=d1[:], in0=d1[:], in1=flag[:], op=OP.mult)
        nc.vector.tensor_tensor(out=lo[:], in0=lo[:], in1=d1[:], op=OP.add)
        nf = g.sb_bis.tile([1, 1], F32, name="bnf", tag="bnf")
        nc.vector.tensor_scalar(out=nf[:], in0=flag[:], scalar1=-1.0, scalar2=1.0,
                                op0=OP.mult, op1=OP.add)
        d2 = g.sb_bis.tile([1, 1], F32, name="bd2", tag="bd2")
        nc.vector.tensor_tensor(out=d2[:], in0=t[:], in1=hi[:], op=OP.subtract)
        nc.vector.tensor_tensor(out=d2[:], in0=d2[:], in1=nf[:], op=OP.mult)
        nc.vector.tensor_tensor(out=hi[:], in0=hi[:], in1=d2[:], op=OP.add)
    return bcast_scalar(g, lo, tag)


def load_full_from_ag(g, ag_dram, tag, nvec=1, vec=0):
    """AG out dram [(8*nvec*128), bpc] -> [128, BT] sbuf."""
    nc = g.nc
    out = g.sb_nv.tile([P, g.BT], F32, name=tag, tag=tag)
    for r in range(NCORES):
        src = ag_dram.ap().rearrange("(r v p) b -> r v p b", v=nvec, p=P)[r, vec]
        nc.sync.dma_start(out=out[:, r * g.bpc:(r + 1) * g.bpc], in_=src)
    return out


def nv(g, tag, shape=None):
    return g.sb_nv.tile(shape or [P, g.bpc], F32, name=tag, tag=tag)


def sel_from(g, score, active, tag):
    """sel = score*active + (active-1)*1e30 (elementwise, any width)."""
    nc = g.nc
    t1 = nv(g, tag, [P, score.shape[-1]])
    nc.vector.tensor_tensor(out=t1[:], in0=score[:], in1=active[:], op=OP.mult)
    t2 = nv(g, tag + "_m", [P, score.shape[-1]])
    nc.vector.tensor_scalar(out=t2[:], in0=active[:], scalar1=1e30,
                            scalar2=-1e30, op0=OP.mult, op1=OP.add)
    nc.vector.tensor_tensor(out=t1[:], in0=t1[:], in1=t2[:], op=OP.add)
    return t1


def build_kernel(cfg, meta):
    g = G()
    g.D = cfg.D
    g.half = cfg.half
    g.bpc = cfg.blocks_per_core
    g.BT = cfg.N_pad // P
    npc = cfg.nodes_per_core
    Np = cfg.N_pad

    nc = bacc.Bacc(trn_type="TRN2")
    g.nc = nc

    cs, rs = meta["cs"], meta["rs"]
    maxtg = 0
    for d in (cs, rs):
        for grp in d["groups"]:
            for h in (0, 1):
                maxtg = max(maxtg, sum(int(d["T"][b][h]) for b in grp))
    g.maxtg = int(maxtg)

    def par(name, shape, dt=F32):
        return nc.declare_dram_parameter(name, list(shape), dt, isOutput=False)

    xs0p = par("xs0", [npc, cfg.D], BF16)       # per-core shard
    g.dirs = {}
    for nm, d in (("cs", cs), ("rs", rs)):
        g.dirs[nm] = dict(d)
        g.dirs[nm]["idx16_d"] = par(f"idx16_{nm}", [128, d["E_flat"] // 16], I16)
        g.dirs[nm]["dstl_d"] = par(f"dstl_{nm}", [128, d["ntiles"]], BF16)
    statc = par("statc", [P, g.bpc, 6])     # u0,q0,r0,aggr0,realmask_sh,pad
    sig0p = par("sigma0", [1, npc], BF16)
    rmfp = par("realmask_full", [P, g.BT])
    iotap = par("iotasmall", [P, P], BF16)
    identp = par("ident", [P, P])
    onesp = par("ones", [P, 2])
    pcolp = par("pcol", [P, 2], BF16)       # p0,p1 as feature columns
    wallp = par("wall", [10 * P, P], BF16)
    ballp = par("ball", [1, 10 * P], BF16)
    out_p = nc.declare_dram_parameter("out", [npc, cfg.D], F32, isOutput=True)

    def dram(name, shape, dt=BF16, shared=False):
        return nc.dram_tensor(name, list(shape), dt,
                              addr_space="Shared" if shared else "Local")

    xs0sh = dram("xs0sh", [npc, cfg.D])
    XS = {k: dram(f"xst{k}", [Np, cfg.D], shared=True) for k in range(0, 14)}
    xsout = {k: dram(f"xso{k}", [npc, cfg.D]) for k in range(1, 14)}
    g.z_dram = dram("ztab", [Np, ZW])
    h2save = dram("h2save", [npc, cfg.D])
    h5save = dram("h5save", [npc, cfg.D])
    h3tmp = dram("h3tmp", [npc, cfg.D])
    h6tmp = dram("h6tmp", [npc, cfg.D])
    score_sh_d = {i: dram(f"scsh{i}", [P, g.bpc], F32) for i in (0, 1)}
    score_fl_d = {i: dram(f"scfl{i}", [NCORES * P, g.bpc], F32, shared=True)
                  for i in (0, 1)}
    uq_sh_d = dram("uqsh", [2 * P, g.bpc], F32)
    uq_fl_d = dram("uqfl", [NCORES * 2 * P, g.bpc], F32, shared=True)
    u2_sh_d = dram("u2sh", [P, g.bpc], F32)
    u2_fl_d = dram("u2fl", [NCORES * P, g.bpc], F32, shared=True)

    with tile.TileContext(nc) as tc:
        g.tc = tc
        ctxs = [
            tc.tile_pool(name="const", bufs=1),
            tc.tile_pool(name="nvp", bufs=1),
            tc.tile_pool(name="idxp", bufs=2),
            tc.tile_pool(name="gathp", bufs=2),
            tc.tile_pool(name="ohtp", bufs=2),
            tc.tile_pool(name="evp", bufs=3),
            tc.tile_pool(name="outp", bufs=3),
            tc.tile_pool(name="zstp", bufs=2),
            tc.tile_pool(name="bisp", bufs=1),
            tc.tile_pool(name="psagg", bufs=3, space="PSUM"),
            tc.tile_pool(name="psw", bufs=2, space="PSUM"),
            tc.tile_pool(name="pst", bufs=2, space="PSUM"),
        ]
        cpool, g.sb_nv, g.sb_idx, g.sb_gath, g.sb_oht, g.sb_ev, g.sb_out, \
            g.sb_zst, g.sb_bis, g.ps_agg, g.ps_w, g.ps_t = \
            [c.__enter__() for c in ctxs]

        def cload(ap_src, shape, tag, dt=F32):
            t = cpool.tile(list(shape), dt, name=tag, tag=tag)
            nc.sync.dma_start(out=t[:], in_=ap_src)
            return t

        iotas = cload(iotap.ap(), [P, P], "iotas", BF16)
        g.iota_m = cpool.tile([P, P, g.maxtg], BF16, name="iotam", tag="iotam")
        nc.vector.tensor_copy(
            out=g.iota_m[:], in_=iotas[:, :, None].to_broadcast([P, P, g.maxtg]))
        g.ident = cload(identp.ap(), [P, P], "ident")
        g.identb = cpool.tile([P, P], BF16, name="identb", tag="identb")
        nc.vector.tensor_copy(out=g.identb[:], in_=g.ident[:])
        ones2 = cload(onesp.ap(), [P, 2], "ones2")
        g.ones_col = ones2[:, 0:1]
        orow = cpool.tile([1, P], F32, name="orow", tag="orow")
        nc.vector.memset(orow[:], 1.0)
        g.ones_row = orow
        statc_t = cload(statc.ap(), [P, g.bpc, 6], "statc")
        u0c = statc_t[:, :, 0]
        q0c = statc_t[:, :, 1]
        r0c = statc_t[:, :, 2]
        aggr0c = statc_t[:, :, 3]
        rm_sh = statc_t[:, :, 4]
        sig0 = cload(sig0p.ap(), [1, npc], "sig0", BF16)
        rm_fl = cload(rmfp.ap(), [P, g.BT], "rmfl")
        pcol_t = cload(pcolp.ap(), [P, 2], "pcol", BF16)
        p0c, p1c = pcol_t[:, 0:1], pcol_t[:, 1:2]
        wall = cload(wallp.ap().rearrange("(w p) d -> p w d", p=P), [P, 10, P],
                     "wall", BF16)
        ball = cload(ballp.ap().rearrange("o (w d) -> o w d", d=P), [1, 10, P],
                     "ball", BF16)
        Wt = [wall[:, i, :] for i in range(10)]
        bt = [ball[:, i, :] for i in range(10)]

        def vmul(a, b_, tag):
            t = nv(g, tag)
            nc.vector.tensor_tensor(out=t[:], in0=a[:], in1=b_[:], op=OP.mult)
            return t

        u0u0 = vmul(u0c, u0c, "u0u0")
        u0q0 = vmul(u0c, q0c, "u0q0")

        def conv_pass(widx, src, sigma_row, outs):
            emit_dir_pass(g, "cs", src, "conv",
                          conv_block_fn(g, Wt[widx], bt[widx], sigma_row),
                          outs=outs)

        def rescale_pass(src_dram, dst_dram, cv):
            """dst = src * cv (per-node col scalar), batched in block chunks."""
            for b0 in range(0, g.bpc, 7):
                nb = min(7, g.bpc - b0)
                t = g.sb_out.tile([P, nb, P], BF16, name="rsx", tag="rsx")
                nc.sync.dma_start(
                    out=t[:],
                    in_=src_dram.ap().rearrange("(b p) d -> p b d", p=P)[:, b0:b0 + nb])
                for j in range(nb):
                    nc.vector.tensor_scalar(
                        out=t[:, j, :], in0=t[:, j, :],
                        scalar1=cv[:, b0 + j: b0 + j + 1], scalar2=None, op0=OP.mult)
                nc.sync.dma_start(
                    out=dst_dram.ap().rearrange("(b p) d -> p b d", p=P)[:, b0:b0 + nb],
                    in_=t[:])

        def emit_schedule():
            # stage xs0 shard into internal dram, AG to full table
            nc.sync.dma_start(out=xs0sh.ap(), in_=xs0p.ap())
            allgather(g, xs0sh, XS[0])
            # =========== DOWN LEVEL 0 ===========
            conv_pass(0, XS[0], sig0, [("xs", BF16, u0u0, xsout[1])])
            allgather(g, xsout[1], XS[1])
            conv_pass(1, XS[1], sig0, [("xs", BF16, u0q0, xsout[2]),
                                       ("xs", BF16, u0c, h2save)])
            allgather(g, xsout[2], XS[2])
            score0 = nv(g, "score0")
            emit_dir_pass(g, "cs", XS[2], "wec",
                          wec_block_fn(g, score_to=score0, colfac=r0c, pcol=p0c),
                          outs=[("xs", BF16, r0c, h3tmp)])
            nc.sync.dma_start(out=score_sh_d[0].ap(), in_=score0[:])
            allgather(g, score_sh_d[0], score_fl_d[0])
            sc0f = load_full_from_ag(g, score_fl_d[0], "sc0f")
            sel0f = sel_from(g, sc0f, rm_fl, "sel0f")
            k0 = math.ceil(cfg.ratio * cfg.N)
            thr0 = bisect_topk(g, sel0f, k0, "thr0")
            kept0f = nv(g, "kept0f", [P, g.BT])
            nc.vector.tensor_scalar(out=kept0f[:], in0=sel0f[:], scalar1=thr0[:],
                                    scalar2=None, op0=OP.is_ge)
            sel0s = sel_from(g, score0, rm_sh, "sel0s")
            kept0s = nv(g, "kept0s")
            nc.vector.tensor_scalar(out=kept0s[:], in0=sel0s[:], scalar1=thr0[:],
                                    scalar2=None, op0=OP.is_ge)
            tanh0 = nv(g, "tanh0")
            nc.scalar.activation(out=tanh0[:], in_=score0[:], func=AF.Tanh)
            # Z pass A: deg1 raw (rs direction: gather kept0 at col, segsum at row)
            zbuild(g, [kept0f])
            S1 = nv(g, "S1")
            emit_dir_pass(g, "rs", None, "z", z_block_fn(g, [], [(S1, 0)], 1),
                          zcols=1)
            deg1 = vmul(kept0s, S1, "deg1")
            m1 = nv(g, "m1")
            nc.vector.tensor_scalar(out=m1[:], in0=deg1[:], scalar1=0.0, scalar2=None,
                                    op0=OP.is_gt)
            dsafe = nv(g, "dsafe")
            nc.vector.tensor_scalar(out=dsafe[:], in0=deg1[:], scalar1=1e-30,
                                    scalar2=None, op0=OP.max)
            u1 = nv(g, "u1")
            nc.vector.reciprocal(out=u1[:], in_=dsafe[:])
            nc.scalar.activation(out=u1[:], in_=u1[:], func=AF.Sqrt)
            nc.vector.tensor_tensor(out=u1[:], in0=u1[:], in1=m1[:], op=OP.mult)
            w1 = vmul(aggr0c, kept0s, "w1")
            rdeg1 = nv(g, "rdeg1")
            nc.vector.reciprocal(out=rdeg1[:], in_=dsafe[:])
            q1 = vmul(w1, rdeg1, "q1")
            nc.vector.tensor_tensor(out=q1[:], in0=q1[:], in1=m1[:], op=OP.mult)
            nc.sync.dma_start(out=uq_sh_d.ap()[0:P], in_=u1[:])
            nc.sync.dma_start(out=uq_sh_d.ap()[P:], in_=q1[:])
            allgather(g, uq_sh_d, uq_fl_d)
            u1f = load_full_from_ag(g, uq_fl_d, "u1f", nvec=2, vec=0)
            q1f = load_full_from_ag(g, uq_fl_d, "q1f", nvec=2, vec=1)
            # Z pass B: sigma1 (row), A1 (col)  (cs direction, gather at row)
            zbuild(g, [u1f, q1f])
            sig1 = cpool.tile([1, npc], BF16, name="sig1", tag="sig1")
            A1 = nv(g, "A1")
            emit_dir_pass(g, "cs", None, "z",
                          z_block_fn(g, [(sig1, 0)], [(A1, 1)], 2), zcols=2)
            aggr1 = vmul(kept0s, A1, "aggr1")
            nc.vector.tensor_scalar(out=aggr1[:], in0=aggr1[:], scalar1=1e-12,
                                    scalar2=None, op0=OP.add)
            raggr1 = nv(g, "raggr1")
            nc.vector.reciprocal(out=raggr1[:], in_=aggr1[:])
            r1 = vmul(kept0s, raggr1, "r1")
            cv3 = vmul(tanh0, u1, "cv3")
            rescale_pass(h3tmp, xsout[3], cv3)
            allgather(g, xsout[3], XS[3])

            # =========== DOWN LEVEL 1 ===========
            u1u1 = vmul(u1, u1, "u1u1")
            u1q1 = vmul(u1, q1, "u1q1")
            conv_pass(2, XS[3], sig1, [("xs", BF16, u1u1, xsout[4])])
            allgather(g, xsout[4], XS[4])
            conv_pass(3, XS[4], sig1, [("xs", BF16, u1q1, xsout[5]),
                                       ("xs", BF16, u1, h5save)])
            allgather(g, xsout[5], XS[5])
            score1 = nv(g, "score1")
            emit_dir_pass(g, "cs", XS[5], "wec",
                          wec_block_fn(g, score_to=score1, colfac=r1, pcol=p1c),
                          outs=[("xs", BF16, r1, h6tmp)])
            nc.sync.dma_start(out=score_sh_d[1].ap(), in_=score1[:])
            allgather(g, score_sh_d[1], score_fl_d[1])
            sc1f = load_full_from_ag(g, score_fl_d[1], "sc1f")
            sel1f = sel_from(g, sc1f, kept0f, "sel1f")
            k1 = math.ceil(cfg.ratio * k0)
            thr1 = bisect_topk(g, sel1f, k1, "thr1")
            kept1f = nv(g, "kept1f", [P, g.BT])
            nc.vector.tensor_scalar(out=kept1f[:], in0=sel1f[:], scalar1=thr1[:],
                                    scalar2=None, op0=OP.is_ge)
            sel1s = sel_from(g, score1, kept0s, "sel1s")
            kept1s = nv(g, "kept1s")
            nc.vector.tensor_scalar(out=kept1s[:], in0=sel1s[:], scalar1=thr1[:],
                                    scalar2=None, op0=OP.is_ge)
            tanh1 = nv(g, "tanh1")
            nc.scalar.activation(out=tanh1[:], in_=score1[:], func=AF.Tanh)
            # Z pass C: deg2 raw
            zbuild(g, [kept1f])
            S2 = nv(g, "S2")
            emit_dir_pass(g, "rs", None, "z", z_block_fn(g, [], [(S2, 0)], 1),
                          zcols=1)
            deg2 = vmul(kept1s, S2, "deg2")
            m2 = nv(g, "m2")
            nc.vector.tensor_scalar(out=m2[:], in0=deg2[:], scalar1=0.0, scalar2=None,
                                    op0=OP.is_gt)
            d2safe = nv(g, "d2safe")
            nc.vector.tensor_scalar(out=d2safe[:], in0=deg2[:], scalar1=1e-30,
                                    scalar2=None, op0=OP.max)
            u2 = nv(g, "u2")
            nc.vector.reciprocal(out=u2[:], in_=d2safe[:])
            nc.scalar.activation(out=u2[:], in_=u2[:], func=AF.Sqrt)
            nc.vector.tensor_tensor(out=u2[:], in0=u2[:], in1=m2[:], op=OP.mult)
            nc.sync.dma_start(out=u2_sh_d.ap(), in_=u2[:])
            allgather(g, u2_sh_d, u2_fl_d)
            u2f = load_full_from_ag(g, u2_fl_d, "u2f")
            # Z pass D: sigma2 (row only)
            zbuild(g, [u2f])
            sig2 = cpool.tile([1, npc], BF16, name="sig2", tag="sig2")
            emit_dir_pass(g, "cs", None, "z", z_block_fn(g, [(sig2, 0)], [], 1),
                          zcols=1)
            cv6 = vmul(tanh1, u2, "cv6")
            rescale_pass(h6tmp, xsout[6], cv6)
            allgather(g, xsout[6], XS[6])

            # =========== BOTTOM ===========
            u2u2 = vmul(u2, u2, "u2u2")
            u2r1 = vmul(u2, r1, "u2r1")
            conv_pass(4, XS[6], sig2, [("xs", BF16, u2u2, xsout[7])])
            allgather(g, xsout[7], XS[7])
            conv_pass(5, XS[7], sig2, [("xs", BF16, u2r1, xsout[8])])
            allgather(g, xsout[8], XS[8])

            # =========== UP LEVEL (emask1): wec-up + 2 convs ===========
            q1u1 = vmul(q1, u1, "q1u1")
            emit_dir_pass(g, "rs", XS[8], "wec", wec_block_fn(g),
                          outs=[("xs", BF16, q1u1, xsout[9])])
            allgather(g, xsout[9], XS[9])
            conv_pass(6, XS[9], sig1, [("xs", BF16, u1u1, xsout[10])])
            allgather(g, xsout[10], XS[10])
            conv_pass(7, XS[10], sig1,
                      [("add", BF16, u1, xsout[11], h5save, r0c)])
            allgather(g, xsout[11], XS[11])

            # =========== UP LEVEL (emask0) ===========
            q0u0 = vmul(q0c, u0c, "q0u0")
            emit_dir_pass(g, "rs", XS[11], "wec", wec_block_fn(g),
                          outs=[("xs", BF16, q0u0, xsout[12])])
            allgather(g, xsout[12], XS[12])
            conv_pass(8, XS[12], sig0, [("xs", BF16, u0u0, xsout[13])])
            allgather(g, xsout[13], XS[13])
            conv_pass(9, XS[13], sig0,
                      [("add", F32, u0c, out_p, h2save, None)])

        emit_schedule()

        for c in reversed(ctxs):
            c.__exit__(None, None, None)

    nc.compile()
    return nc


def make_inmaps(cfg, meta):
    cs, rs = meta["cs"], meta["rs"]
    bpc = cfg.blocks_per_core
    npc = cfg.nodes_per_core
    sv = meta["shardvec"]
    u0s, q0s, r0s, ag0s = (sv(meta[k]) for k in ("u0", "q0", "r0", "aggr0"))
    rms = sv(meta["realmask"])
    sig0s = meta["sigma0"].reshape(NCORES, npc).astype(bfloat16)
    rmf = meta["fullvec"](meta["realmask"])
    iotas = np.tile(np.arange(P, dtype=np.float32)[None, :], (P, 1)).astype(bfloat16)
    ident = np.eye(P, dtype=np.float32)
    ones = np.ones((P, 2), np.float32)
    pcol = np.stack([meta["p0"], meta["p1"]], 1).astype(bfloat16)
    xs0b = meta["xs0"].astype(bfloat16)

    in_maps = []
    for c in range(NCORES):
        statcv = np.zeros((P, bpc, 6), np.float32)
        statcv[:, :, 0] = u0s[c]
        statcv[:, :, 1] = q0s[c]
        statcv[:, :, 2] = r0s[c]
        statcv[:, :, 3] = ag0s[c]
        statcv[:, :, 4] = rms[c]
        in_maps.append({
            "xs0": xs0b[c * npc:(c + 1) * npc],
            "idx16_cs": cs["idx16"][c], "dstl_cs": cs["dstl"][c],
            "idx16_rs": rs["idx16"][c], "dstl_rs": rs["dstl"][c],
            "statc": statcv, "sigma0": sig0s[c][None, :],
            "realmask_full": rmf,
            "iotasmall": iotas, "ident": ident, "ones": ones, "pcol": pcol,
            "wall": None, "ball": None,
        })
    return in_maps


def fill_weights(in_maps, Wd, bd, Wu, bu, Wb, bb):
    Ws = [Wd[0, 0], Wd[0, 1], Wd[1, 0], Wd[1, 1], Wb[0], Wb[1],
          Wu[0, 0], Wu[0, 1], Wu[1, 0], Wu[1, 1]]
    bs = [bd[0, 0], bd[0, 1], bd[1, 0], bd[1, 1], bb[0], bb[1],
          bu[0, 0], bu[0, 1], bu[1, 0], bu[1, 1]]
    wall = np.concatenate([w.astype(np.float32) for w in Ws], 0).astype(bfloat16)
    ball = np.stack([b.astype(np.float32) for b in bs], 0).astype(bfloat16)
    for m in in_maps:
        m["wall"] = wall
        m["ball"] = ball.reshape(1, -1)


def run_gnn(cfg, inputs, nc_cache={}, full_pad=False, trace=False):
    """Full pipeline: preprocess, build (cached by cfg), run, assemble."""
    x = np.asarray(inputs["x"], np.float32)
    ei = np.asarray(inputs["edge_index"])
    pvec = np.asarray(inputs["pvec"], np.float32)
    meta = preprocess(cfg, x, ei, pvec)
    key = (cfg.N, cfg.E, ei.tobytes()[:64])  # program depends on edge stats
    if key not in nc_cache:
        nc_cache.clear()
        nc_cache[key] = (build_kernel(cfg, meta), None)
    nc, _ = nc_cache[key]
    in_maps = make_inmaps(cfg, meta)
    fill_weights(in_maps, *(np.asarray(inputs[k], np.float32)
                            for k in ("Wd", "bd", "Wu", "bu", "Wb", "bb")))
    res = run_bass_kernel_spmd(nc, in_maps, list(range(NCORES)), trace=trace)
    out = np.concatenate([res.results[c]["out"] for c in range(NCORES)], 0)
    return (out if full_pad else out[: cfg.N]), res


_CFG = Cfg()


def kernel(**inputs):
    out, _ = run_gnn(_CFG, inputs)
    return out.astype(np.float32)


# revision 46
# speedup vs baseline: 1.6573x; 1.6573x over previous
"""Trainium2 Bass kernel for nn_MessagePassingLayer (graph U-Net message
passing) on 8 NeuronCores.

Self-contained: kernel(**inputs) takes the full unsharded inputs and
returns the full [50000, 128] float32 output.

Strategy: nodes padded to 50176 and sharded contiguously over the 8
cores; edges bucketed per (dst block, src half) in destination-sorted
order. Every per-edge coefficient factorizes as rowfac[row] * colfac[col],
so row factors are pre-applied to the gathered node table (bf16 XS
buffers, exchanged via AllGather) and col factors are applied on PSUM
eviction. Edge pipeline runs in bf16: dma_gather of 256B rows, one-hot
tiles in transposed [128, dst, tile] layout (DVE 2x mode) + PE matmul
accumulation per 128-dst block, then W matmul + rank-1 bias + PE
transpose. Top-k pooling is an on-device replicated fp32 threshold
bisection; degree/weight renormalization uses narrow bf16 Z-table
gather passes.
"""
import math

import numpy as np
from ml_dtypes import bfloat16

import concourse.bacc as bacc
import concourse.mybir as mybir
import concourse.tile as tile
from concourse.bass_utils import run_bass_kernel_spmd

from dataclasses import dataclass, field

P = 128
NCORES = 8
GROUP_BLOCKS = 7


@dataclass
class Cfg:
    N: int = 50000
    E: int = 800000
    D: int = 128
    L: int = 2
    B: int = 2
    ratio: float = 0.5
    blocks_per_core: int = field(init=False)
    N_pad: int = field(init=False)
    nodes_per_core: int = field(init=False)
    half: int = field(init=False)          # rows per gather half-table

    def __post_init__(self):
        blocks_total = math.ceil(self.N / P)
        self.blocks_per_core = math.ceil(blocks_total / NCORES)
        self.N_pad = self.blocks_per_core * NCORES * P
        self.nodes_per_core = self.blocks_per_core * P
        self.half = self.N_pad // 2
        assert self.half <= 32768, "gather half-table must fit int16 index"
        assert self.half % P == 0


def wrap_idx(idx, n):
    """[n] int -> [128, n/16] int16 wrapped+replicated layout for dma_gather."""
    assert n % 16 == 0
    w = np.zeros((16, n // 16), np.int16)
    w[np.arange(n) % 16, np.arange(n) // 16] = idx.astype(np.int16)
    return np.tile(w, (8, 1))


def build_direction(cfg, src, dst, group_blocks):
    """Static tables for one scatter direction (gather at src, scatter to dst).

    Edges bucketed per (core, local dst block, src half), padded to tiles
    of 128 with null edges. Tile counts per (block, half) are maxed across
    cores so the 8 cores share one instruction stream; per (group, half)
    tile sums are padded to even so one-hot tiles stay 4B-aligned in bf16.
    """
    bpc = cfg.blocks_per_core
    npc = cfg.nodes_per_core
    half = cfg.half
    zero_row = cfg.N_pad - 1          # pad node: always-zero row (half 1)

    buckets = [[[None, None] for _ in range(bpc)] for _ in range(NCORES)]
    core_of = dst // npc
    blk_of = (dst % npc) // P
    half_of = (src >= half).astype(np.int64)
    order = np.lexsort((src, half_of, blk_of, core_of))
    key = ((core_of[order] * bpc) + blk_of[order]) * 2 + half_of[order]
    bounds = np.searchsorted(key, np.arange(NCORES * bpc * 2 + 1))
    for c in range(NCORES):
        for b in range(bpc):
            for h in (0, 1):
                kk = (c * bpc + b) * 2 + h
                buckets[c][b][h] = order[bounds[kk]:bounds[kk + 1]]

    T = np.zeros((bpc, 2), np.int64)
    for b in range(bpc):
        for h in (0, 1):
            mx = max(len(buckets[c][b][h]) for c in range(NCORES))
            T[b, h] = max(1, math.ceil(mx / P))

    groups = []
    for g0 in range(0, bpc, group_blocks):
        groups.append(list(range(g0, min(g0 + group_blocks, bpc))))

    # pad per-(group, half) tile counts to even (bump last block in group)
    for grp in groups:
        for h in (0, 1):
            if sum(int(T[b, h]) for b in grp) % 2:
                T[grp[-1], h] += 1

    tot_tiles = int(T.sum())
    E_flat = tot_tiles * P
    idxs = np.zeros((NCORES, E_flat), np.int64)
    dstl = np.full((NCORES, E_flat), 200.0, np.float32)
    pos = 0
    tile_plan = []   # (h, b, ntiles, start_pos) shared across cores
    for grp in groups:
        for h in (0, 1):
            for b in grp:
                nt = int(T[b, h])
                tile_plan.append((h, b, nt, pos))
                for c in range(NCORES):
                    ed = buckets[c][b][h]
                    n = len(ed)
                    assert n <= nt * P
                    idxs[c, pos:pos + n] = src[ed] - h * half
                    dstl[c, pos:pos + n] = (dst[ed] % npc) % P
                    # null edges: h=1 gathers the pad zero row; h=0 gathers
                    # real row 0 but dstl=200 never matches the iota
                    if n < nt * P:
                        idxs[c, pos + n:pos + nt * P] = (zero_row - half) if h else 0
                pos += nt * P
    assert pos == E_flat

    # wrapped idx in 16 partitions only; replicated to 128 on device
    idx16 = np.zeros((NCORES, 16, E_flat // 16), np.int16)
    for c in range(NCORES):
        idx16[c, np.arange(E_flat) % 16, np.arange(E_flat) // 16] = \
            idxs[c].astype(np.int16)
    ntiles = E_flat // P
    dstl_t = dstl.reshape(NCORES, ntiles, P).transpose(0, 2, 1).astype(bfloat16)

    return {
        "idx16": idx16,            # [NCORES, 128, E_flat/16] int16
        "dstl": dstl_t,            # [NCORES, 128, ntiles] bf16
        "tile_plan": tile_plan,
        "groups": groups,
        "E_flat": E_flat,
        "ntiles": ntiles,
        "T": T,
    }


def preprocess(cfg, x, edge_index, pvec):
    """All static host work. Returns per-core input pieces + meta."""
    N, Np = cfg.N, cfg.N_pad
    row = edge_index[0].astype(np.int64)
    col = edge_index[1].astype(np.int64)

    deg0 = np.zeros(Np, np.float32)
    np.add.at(deg0, row, 1.0)
    with np.errstate(divide="ignore"):
        dis0 = np.where(deg0 > 0, deg0.astype(np.float64) ** -0.5, 0.0).astype(np.float32)
    normed0 = np.where(deg0 > 0, 1.0 / np.where(deg0 > 0, deg0, 1), 0.0).astype(np.float32)
    A0 = np.zeros(Np, np.float32)
    np.add.at(A0, col, normed0[row])
    aggr0 = (A0 + 1e-12).astype(np.float32)
    r0 = (1.0 / aggr0).astype(np.float32)
    q0 = normed0
    u0 = dis0

    cs = build_direction(cfg, row, col, GROUP_BLOCKS)
    rs = build_direction(cfg, col, row, GROUP_BLOCKS)

    realmask = np.zeros(Np, np.float32)
    realmask[:N] = 1.0

    p0 = pvec[0] / np.linalg.norm(pvec[0])
    p1 = pvec[1] / np.linalg.norm(pvec[1])

    def shardvec(v):   # [Np] -> [NCORES, 128, bpc]
        return v.reshape(NCORES, cfg.blocks_per_core, P).transpose(0, 2, 1).copy()

    def fullvec(v):    # [Np] -> [128, blocks_total]
        return v.reshape(-1, P).T.copy()

    return {
        "cs": cs, "rs": rs, "x": x,
        "u0": u0, "q0": q0, "r0": r0, "aggr0": aggr0,
        "realmask": realmask, "p0": p0, "p1": p1,
        "shardvec": shardvec, "fullvec": fullvec,
    }


F32 = mybir.dt.float32
BF16 = mybir.dt.bfloat16
I16 = mybir.dt.int16
AF = mybir.ActivationFunctionType
OP = mybir.AluOpType
AX = mybir.AxisListType

ZCHUNK = 16
ZW = 128          # bf16 z rows must be 256B for dma_gather
BISECT_ITERS = 30


class G:
    """build-time globals bag"""
    pass


# ------------------------------------------------------------- edge passes --

def emit_dir_pass(g, dirn, src_dram, mode, block_fn, outs=(), zcols=0):
    """One edge pass.

    mode: 'conv' / 'wec' (psum [feat, dst], block_fn returns final psum
    [dst, feat] for eviction) or 'z' (psum [zcols, dst], no eviction).
    outs: eviction specs applied per group with batched DMA.
    """
    nc = g.nc
    d = g.dirs[dirn]
    elem = ZW if mode == "z" else g.D
    if mode == "z":
        in_aps = [g.z_dram.ap()[0:g.half, :], g.z_dram.ap()[g.half:, :]]
    else:
        in_aps = [src_dram.ap()[0:g.half, :], src_dram.ap()[g.half:, :]]

    gtag = "fg"
    plan = {(h, b): (nt, pos) for (h, b, nt, pos) in d["tile_plan"]}
    NB = GROUP_BLOCKS
    for grp in d["groups"]:
        t0g = plan[(0, grp[0])][1] // P
        ntg_h = [sum(plan[(h, b)][0] for b in grp) for h in (0, 1)]
        ntg_all = ntg_h[0] + ntg_h[1]
        it = g.sb_idx.tile([128, 2 * g.maxtg * 8], I16, name="idx", tag="idx")
        nc.sync.dma_start(out=it[:, :ntg_all * 8],
                          in_=d["idx16_d"].ap()[:, t0g * 8: t0g * 8 + ntg_all * 8])
        dlg = g.sb_idx.tile([128, 2 * g.maxtg], BF16, name="dl", tag="dl")
        nc.sync.dma_start(out=dlg[:, :ntg_all],
                          in_=d["dstl_d"].ap()[:, t0g: t0g + ntg_all])
        parts = {}
        for h in (0, 1):
            off = 0 if h == 0 else ntg_h[0]
            ntg = ntg_h[h]
            gt = g.sb_gath.tile([P, g.maxtg, elem], BF16, name="fg", tag=gtag)
            nc.gpsimd.dma_gather(
                out_ap=gt[:, :ntg, :], in_ap=in_aps[h],
                idxs_ap=it[:, off * 8: off * 8 + ntg * 8],
                num_idxs=ntg * P, num_idxs_reg=ntg * P, elem_size=elem,
                single_packet=False)
            oh = g.sb_oht.tile([P, P, g.maxtg], BF16, name="oht", tag="oht")
            nc.vector.tensor_tensor(
                out=oh[:, :, :ntg],
                in0=dlg[:, None, off: off + ntg].to_broadcast([P, P, ntg]),
                in1=g.iota_m[:, :, :ntg], op=OP.is_equal)
            parts[h] = (gt, oh)
        gbufs = [g.sb_out.tile([P, NB, P], o[1], name="gb", tag=f"gb{o[1]}{i}")
                 for i, o in enumerate(outs)]
        # per-block interleaved halves so only a couple agg psums are live
        cum = {0: 0, 1: 0}
        for j, b in enumerate(grp):
            psum = g.ps_agg.tile([P, P], F32, space="PSUM", name="agg", tag="agg")
            tot = plan[(0, b)][0] + plan[(1, b)][0]
            k = 0
            for h in (0, 1):
                gt, oh = parts[h]
                nt, _pos = plan[(h, b)]
                tpos = cum[h]
                for t in range(nt):
                    k += 1
                    if mode == "z":
                        lhs = gt[:, tpos + t, :zcols]
                        outp = psum[:zcols, :]
                    else:
                        lhs = gt[:, tpos + t, :]
                        outp = psum[:]
                    nc.tensor.matmul(out=outp, lhsT=lhs, rhs=oh[:, :, tpos + t],
                                     start=(k == 1), stop=(k == tot))
                cum[h] += nt
            fin = block_fn(b, psum[:zcols, :] if mode == "z" else psum)
            if fin is not None:
                for gb, o in zip(gbufs, outs):
                    nc.scalar.activation(
                        out=gb[:, j, :], in_=fin[:], func=AF.Copy,
                        scale=o[2][:, b: b + 1])
                    if o[0] == "xs" and len(o) > 4 and o[4] is not None:
                        bbc, bcol = o[4], o[5]
                        bt_ = g.sb_ev.tile([P, P], BF16, name="bt", tag="bt")
                        nc.scalar.activation(out=bt_[:], in_=bbc[:], func=AF.Copy,
                                             scale=bcol[:, b: b + 1])
                        nc.vector.tensor_tensor(out=gb[:, j, :], in0=gb[:, j, :],
                                                in1=bt_[:], op=OP.add)
        # flush group evictions
        b0 = grp[0]
        nb = len(grp)
        for gb, o in zip(gbufs, outs):
            kind, dt_ = o[0], o[1]
            dst_dram = o[3]
            dview = dst_dram.ap().rearrange("(b p) d -> p b d", p=P)[:, b0: b0 + nb]
            if kind == "xs":
                nc.sync.dma_start(out=dview, in_=gb[:, :nb, :])
            else:  # add: (kind, dtype, pre, out_dram, in_dram, post)
                in_dram, post = o[4], o[5]
                sk = g.sb_out.tile([P, NB, P], BF16, name="skl", tag="skl")
                nc.sync.dma_start(
                    out=sk[:, :nb, :],
                    in_=in_dram.ap().rearrange("(b p) d -> p b d", p=P)[:, b0: b0 + nb])
                nc.vector.tensor_tensor(out=gb[:, :nb, :], in0=gb[:, :nb, :],
                                        in1=sk[:, :nb, :], op=OP.add)
                if post is not None:
                    for j, b in enumerate(grp):
                        nc.vector.tensor_scalar(
                            out=gb[:, j, :], in0=gb[:, j, :],
                            scalar1=post[:, b: b + 1], scalar2=None, op0=OP.mult)
                nc.sync.dma_start(out=dview, in_=gb[:, :nb, :])


def agg_block_fn(g, W_sb=None, score_to=None, colfac=None, pcol=None):
    """Aggregation post-processing: optional next-conv W transform, optional
    wec score tap (from the pre-W aggregate), then transpose to [dst, feat].
    """
    nc = g.nc

    def fn(b, pag):
        a1 = g.sb_ev.tile([P, P], BF16, name="a1", tag="a1")
        nc.scalar.activation(out=a1[:], in_=pag[:], func=AF.Copy)
        if score_to is not None:
            psc = g.ps_w.tile([P, P], F32, space="PSUM", name="psc", tag="p2")
            nc.tensor.matmul(out=psc[:, :1], lhsT=a1[:], rhs=pcol[:],
                             start=True, stop=True)
            nc.vector.tensor_tensor(out=score_to[:, b: b + 1], in0=psc[:, :1],
                                    in1=colfac[:, b: b + 1], op=OP.mult)
        src = a1
        if W_sb is not None:
            p2 = g.ps_w.tile([P, P], F32, space="PSUM", name="p2", tag="p2")
            nc.tensor.matmul(out=p2[:], lhsT=W_sb[:], rhs=a1[:],
                             start=True, stop=True)
            a2 = g.sb_ev.tile([P, P], BF16, name="a2", tag="a2")
            nc.scalar.activation(out=a2[:], in_=p2[:], func=AF.Copy)
            src = a2
        p3 = g.ps_t.tile([P, P], BF16, space="PSUM", name="pst", tag="pst")
        nc.tensor.transpose(out=p3[:], in_=src[:], identity=g.identb[:])
        return p3
    return fn


def z_block_fn(g, row_to, col_to, zcols):
    """row_to: [(rowtile, j)] copy psum row j (bf16 cast); col_to: fp32 cols."""
    nc = g.nc

    def fn(b, pag):
        for (rt, j) in row_to:
            nc.vector.tensor_copy(out=rt[:, b * P:(b + 1) * P], in_=pag[j:j + 1, :])
        if col_to:
            az = g.sb_ev.tile([P, P], F32, name="az", tag="az")
            nc.vector.tensor_copy(out=az[:zcols, :], in_=pag[:])
            pz = g.ps_w.tile([P, P], F32, space="PSUM", name="pz", tag="p2")
            nc.tensor.transpose(out=pz[:, :zcols], in_=az[:zcols, :],
                                identity=g.ident[:zcols, :zcols])
            for (ct, j) in col_to:
                nc.vector.tensor_copy(out=ct[:, b: b + 1], in_=pz[:, j: j + 1])
        return None
    return fn


# ------------------------------------------------------------- small pieces --

def allgather(g, in_dram, out_dram):
    g.nc.gpsimd.collective_compute(
        "AllGather", OP.bypass, replica_groups=[list(range(NCORES))],
        ins=[in_dram.ap()], outs=[out_dram.ap()])


def zbuild(g, cols):
    nc = g.nc
    for c0 in range(0, g.BT, ZCHUNK):
        nb = min(ZCHUNK, g.BT - c0)
        st = g.sb_zst.tile([P, ZCHUNK, ZW], BF16, name="zst", tag="zst")
        for j, v in enumerate(cols):
            nc.vector.tensor_copy(out=st[:, :nb, j:j + 1], in_=v[:, c0:c0 + nb, None])
        nc.sync.dma_start(
            out=g.z_dram.ap().rearrange("(b p) w -> p b w", p=P)[:, c0:c0 + nb, :],
            in_=st[:, :nb, :])


def cross_part(g, col, op, tag="bred"):
    nc = g.nc
    if op == "sum":
        pc = g.ps_w.tile([P, P], F32, space="PSUM", name="pc", tag="p2")
        nc.tensor.matmul(out=pc[:1, :1], lhsT=col[:], rhs=g.ones_col[:],
                         start=True, stop=True)
        out = g.sb_bis.tile([1, 1], F32, name="cnt", tag="cnt")
        nc.vector.tensor_copy(out=out[:], in_=pc[:1, :1])
        return out
    pt = g.ps_w.tile([P, P], F32, space="PSUM", name="pt", tag="p2")
    nc.tensor.transpose(out=pt[:1, :], in_=col[:], identity=g.ident[:])
    row = g.sb_bis.tile([1, P], F32, name="brow", tag="brow")
    nc.vector.tensor_copy(out=row[:], in_=pt[:1, :])
    out = g.sb_bis.tile([1, 1], F32, name="bred", tag=tag)
    nc.vector.reduce_max(out[:], row[:], axis=AX.X)
    return out


def bcast_scalar(g, s11, tag):
    nc = g.nc
    pb = g.ps_w.tile([P, P], F32, space="PSUM", name="pb", tag="p2")
    nc.tensor.matmul(out=pb[:, :1], lhsT=g.ones_row[:], rhs=s11[:],
                     start=True, stop=True)
    out = g.sb_nv.tile([P, 1], F32, name=tag, tag=tag)
    nc.vector.tensor_copy(out=out[:], in_=pb[:, :1])
    return out


def bisect_topk(g, sel_full, k, neg_sel, tag):
    """threshold col [128,1] such that count(sel >= t) == k exactly.

    neg_sel is the masked negation of the scores (active: -score, inactive:
    -1e30) so -max(neg_sel) is the min active score -> tight initial range.
    """
    nc = g.nc
    mx = g.sb_bis.tile([P, 1], F32, name="bmx", tag="bmx")
    nc.vector.reduce_max(mx[:], sel_full[:], axis=AX.X)
    hi = cross_part(g, mx, "max", tag="bhi")
    nc.vector.tensor_scalar(out=hi[:], in0=hi[:], scalar1=1.0, scalar2=None,
                            op0=OP.add)
    mn = g.sb_bis.tile([P, 1], F32, name="bmn", tag="bmn")
    nc.vector.reduce_max(mn[:], neg_sel[:], axis=AX.X)
    lo = cross_part(g, mn, "max", tag="blo2")
    nc.vector.tensor_scalar(out=lo[:], in0=lo[:], scalar1=-1.0, scalar2=-1.0,
                            op0=OP.mult, op1=OP.add)
    t = g.sb_bis.tile([1, 1], F32, name="bt", tag="bt")
    for _ in range(BISECT_ITERS):
        nc.vector.tensor_tensor(out=t[:], in0=lo[:], in1=hi[:], op=OP.add)
        nc.vector.tensor_scalar(out=t[:], in0=t[:], scalar1=0.5, scalar2=None,
                                op0=OP.mult)
        tcol = bcast_scalar(g, t, "btc")
        cmp = g.sb_bis.tile([P, g.BT], F32, name="bcmp", tag="bcmp")
        nc.vector.tensor_scalar(out=cmp[:], in0=sel_full[:], scalar1=tcol[:],
                                scalar2=None, op0=OP.is_ge)
        red = g.sb_bis.tile([P, 1], F32, name="bred2", tag="bred2")
        nc.vector.reduce_sum(red[:], cmp[:], axis=AX.X)
        cnt = cross_part(g, red, "sum")
        flag = g.sb_bis.tile([1, 1], F32, name="bflag", tag="bflag")
        nc.vector.tensor_scalar(out=flag[:], in0=cnt[:], scalar1=float(k) - 0.5,
                                scalar2=None, op0=OP.is_ge)
        d1 = g.sb_bis.tile([1, 1], F32, name="bd1", tag="bd1")
        nc.vector.tensor_tensor(out=d1[:], in0=t[:], in1=lo[:], op=OP.subtract)
        nc.vector.tensor_tensor(out=d1[:], in0=d1[:], in1=flag[:], op=OP.mult)
        nc.vector.tensor_tensor(out=lo[:], in0=lo[:], in1=d1[:], op=OP.add)
        nf = g.sb_bis.tile([1, 1], F32, name="bnf", tag="bnf")
        nc.vector.tensor_scalar(out=nf[:], in0=flag[:], scalar1=-1.0, scalar2=1.0,
                                op0=OP.mult, op1=OP.add)
        d2 = g.sb_bis.tile([1, 1], F32, name="bd2", tag="bd2")
        nc.vector.tensor_tensor(out=d2[:], in0=t[:], in1=hi[:], op=OP.subtract)
        nc.vector.tensor_tensor(out=d2[:], in0=d2[:], in1=nf[:], op=OP.mult)
        nc.vector.tensor_tensor(out=hi[:], in0=hi[:], in1=d2[:], op=OP.add)
    return bcast_scalar(g, lo, tag)


def load_full_from_ag(g, ag_dram, tag, nvec=1, vec=0):
    """AG out dram [(8*nvec*128), bpc] -> [128, BT] sbuf."""
    nc = g.nc
    out = g.sb_nv.tile([P, g.BT], F32, name=tag, tag=tag)
    for r in range(NCORES):
        src = ag_dram.ap().rearrange("(r v p) b -> r v p b", v=nvec, p=P)[r, vec]
        nc.sync.dma_start(out=out[:, r * g.bpc:(r + 1) * g.bpc], in_=src)
    return out


def nv(g, tag, shape=None):
    return g.sb_nv.tile(shape or [P, g.bpc], F32, name=tag, tag=tag)


def sel_from(g, score, active, tag):
    """sel = score*active + (active-1)*1e30 (elementwise, any width)."""
    nc = g.nc
    t1 = nv(g, tag, [P, score.shape[-1]])
    nc.vector.tensor_tensor(out=t1[:], in0=score[:], in1=active[:], op=OP.mult)
    t2 = nv(g, tag + "_m", [P, score.shape[-1]])
    nc.vector.tensor_scalar(out=t2[:], in0=active[:], scalar1=1e30,
                            scalar2=-1e30, op0=OP.mult, op1=OP.add)
    nc.vector.tensor_tensor(out=t1[:], in0=t1[:], in1=t2[:], op=OP.add)
    return t1


def build_kernel(cfg, meta):
    g = G()
    g.D = cfg.D
    g.half = cfg.half
    g.bpc = cfg.blocks_per_core
    g.BT = cfg.N_pad // P
    npc = cfg.nodes_per_core
    Np = cfg.N_pad

    nc = bacc.Bacc(trn_type="TRN2")
    g.nc = nc

    cs, rs = meta["cs"], meta["rs"]
    maxtg = 0
    for d in (cs, rs):
        for grp in d["groups"]:
            for h in (0, 1):
                maxtg = max(maxtg, sum(int(d["T"][b][h]) for b in grp))
    g.maxtg = int(maxtg)

    def par(name, shape, dt=F32):
        return nc.declare_dram_parameter(name, list(shape), dt, isOutput=False)

    xs0p = par("xs0", [npc, cfg.D], BF16)       # per-core shard of (xW0+b0)*u0
    g.dirs = {}
    idx_p16 = {}
    for nm, d in (("cs", cs), ("rs", rs)):
        g.dirs[nm] = dict(d)
        idx_p16[nm] = par(f"idx16_{nm}", [16, d["E_flat"] // 16], I16)
        g.dirs[nm]["dstl_d"] = par(f"dstl_{nm}", [128, d["ntiles"]], BF16)
    statc = par("statc", [P, g.bpc, 6])     # u0,q0,r0,aggr0,realmask_sh,pad
    rmfp = par("realmask_full", [P, g.BT])
    iotap = par("iotasmall", [P, P], BF16)
    identp = par("ident", [P, P])
    onesp = par("ones", [P, 2])
    pcolp = par("pcol", [P, 2], BF16)       # p0,p1 as feature columns
    wallp = par("wall", [10 * P, P], BF16)
    ballp = par("ballbc", [P, 10 * P], BF16)   # biases replicated per partition
    out_p = nc.declare_dram_parameter("out", [npc, cfg.D], F32, isOutput=True)

    def dram(name, shape, dt=BF16, shared=False):
        return nc.dram_tensor(name, list(shape), dt,
                              addr_space="Shared" if shared else "Local")

    xs0sh = dram("xs0sh", [npc, cfg.D])
    for nm in ("cs", "rs"):
        g.dirs[nm]["idx16_d"] = nc.dram_tensor(
            f"idx16r_{nm}", [128, g.dirs[nm]["E_flat"] // 16], I16,
            addr_space="Local")
    XS = {k: dram(f"xst{k}", [Np, cfg.D], shared=True) for k in range(0, 14)}
    xsout = {k: dram(f"xso{k}", [npc, cfg.D]) for k in range(1, 14)}
    g.z_dram = dram("ztab", [Np, ZW])
    h2save = dram("h2save", [npc, cfg.D])
    h5save = dram("h5save", [npc, cfg.D])
    h3tmp = dram("h3tmp", [npc, cfg.D])
    h6tmp = dram("h6tmp", [npc, cfg.D])
    score_sh_d = {i: dram(f"scsh{i}", [P, g.bpc], F32) for i in (0, 1)}
    score_fl_d = {i: dram(f"scfl{i}", [NCORES * P, g.bpc], F32, shared=True)
                  for i in (0, 1)}
    q1_sh_d = dram("q1sh", [P, g.bpc], F32)
    q1_fl_d = dram("q1fl", [NCORES * P, g.bpc], F32, shared=True)

    with tile.TileContext(nc) as tc:
        g.tc = tc
        ctxs = [
            tc.tile_pool(name="const", bufs=1),
            tc.tile_pool(name="nvp", bufs=1),
            tc.tile_pool(name="idxp", bufs=2),
            tc.tile_pool(name="gathp", bufs=3),
            tc.tile_pool(name="ohtp", bufs=2),
            tc.tile_pool(name="evp", bufs=3),
            tc.tile_pool(name="outp", bufs=3),
            tc.tile_pool(name="zstp", bufs=1),
            tc.tile_pool(name="bisp", bufs=1),
            tc.tile_pool(name="psagg", bufs=3, space="PSUM"),
            tc.tile_pool(name="psw", bufs=2, space="PSUM"),
            tc.tile_pool(name="pst", bufs=2, space="PSUM"),
        ]
        cpool, g.sb_nv, g.sb_idx, g.sb_gath, g.sb_oht, g.sb_ev, g.sb_out, \
            g.sb_zst, g.sb_bis, g.ps_agg, g.ps_w, g.ps_t = \
            [c.__enter__() for c in ctxs]

        def cload(ap_src, shape, tag, dt=F32):
            t = cpool.tile(list(shape), dt, name=tag, tag=tag)
            nc.sync.dma_start(out=t[:], in_=ap_src)
            return t

        iotas = cload(iotap.ap(), [P, P], "iotas", BF16)
        g.iota_m = cpool.tile([P, P, g.maxtg], BF16, name="iotam", tag="iotam")
        nc.vector.tensor_copy(
            out=g.iota_m[:], in_=iotas[:, :, None].to_broadcast([P, P, g.maxtg]))
        g.ident = cload(identp.ap(), [P, P], "ident")
        g.identb = cpool.tile([P, P], BF16, name="identb", tag="identb")
        nc.vector.tensor_copy(out=g.identb[:], in_=g.ident[:])
        ones2 = cload(onesp.ap(), [P, 2], "ones2")
        g.ones_col = ones2[:, 0:1]
        orow = cpool.tile([1, P], F32, name="orow", tag="orow")
        nc.vector.memset(orow[:], 1.0)
        g.ones_row = orow
        statc_t = cload(statc.ap(), [P, g.bpc, 6], "statc")
        u0c = statc_t[:, :, 0]
        q0c = statc_t[:, :, 1]
        r0c = statc_t[:, :, 2]
        aggr0c = statc_t[:, :, 3]
        rm_sh = statc_t[:, :, 4]
        rm_fl = cload(rmfp.ap(), [P, g.BT], "rmfl")
        pcol_t = cload(pcolp.ap(), [P, 2], "pcol", BF16)
        p0c, p1c = pcol_t[:, 0:1], pcol_t[:, 1:2]
        wall = cload(wallp.ap().rearrange("(w p) d -> p w d", p=P), [P, 10, P],
                     "wall", BF16)
        ballbc = cload(ballp.ap().rearrange("p (w d) -> p w d", d=P), [P, 10, P],
                       "ballbc", BF16)
        Wt = [wall[:, i, :] for i in range(10)]
        bbc = [ballbc[:, i, :] for i in range(10)]

        def vmul(a, b_, tag):
            t = nv(g, tag)
            nc.vector.tensor_tensor(out=t[:], in0=a[:], in1=b_[:], op=OP.mult)
            return t

        u0u0 = vmul(u0c, u0c, "u0u0")
        u0q0 = vmul(u0c, q0c, "u0q0")

        def conv_pass(src, outs, W_next=None, score_to=None, colfac=None,
                      pcol=None, dirn="cs"):
            emit_dir_pass(g, dirn, src, "conv",
                          agg_block_fn(g, W_sb=W_next, score_to=score_to,
                                       colfac=colfac, pcol=pcol),
                          outs=outs)

        def rescale_pass(src_dram, dst_dram, cv, bbc=None, bcol=None):
            """dst = src * cv (+ bias_bc * bcol), batched in block chunks."""
            for b0 in range(0, g.bpc, 7):
                nb = min(7, g.bpc - b0)
                t = g.sb_out.tile([P, 7, P], BF16, name="rsx", tag="rsx")
                nc.sync.dma_start(
                    out=t[:, :nb, :],
                    in_=src_dram.ap().rearrange("(b p) d -> p b d", p=P)[:, b0:b0 + nb])
                for j in range(nb):
                    nc.vector.tensor_scalar(
                        out=t[:, j, :], in0=t[:, j, :],
                        scalar1=cv[:, b0 + j: b0 + j + 1], scalar2=None, op0=OP.mult)
                    if bbc is not None:
                        bt_ = g.sb_ev.tile([P, P], BF16, name="bt", tag="bt")
                        nc.scalar.activation(out=bt_[:], in_=bbc[:], func=AF.Copy,
                                             scale=bcol[:, b0 + j: b0 + j + 1])
                        nc.vector.tensor_tensor(out=t[:, j, :], in0=t[:, j, :],
                                                in1=bt_[:], op=OP.add)
                nc.sync.dma_start(
                    out=dst_dram.ap().rearrange("(b p) d -> p b d", p=P)[:, b0:b0 + nb],
                    in_=t[:, :nb, :])

        def emit_schedule():
            # replicate 16-row wrapped idx tables to 128 partitions in DRAM
            CH = 4096
            for nm in ("cs", "rs"):
                xcols = g.dirs[nm]["E_flat"] // 16
                for c0 in range(0, xcols, CH):
                    w = min(CH, xcols - c0)
                    t16 = g.sb_out.tile([16, CH], I16, name="i16", tag="i16")
                    nc.sync.dma_start(out=t16[:, :w],
                                      in_=idx_p16[nm].ap()[:, c0:c0 + w])
                    for r in range(8):
                        nc.sync.dma_start(
                            out=g.dirs[nm]["idx16_d"].ap()[16 * r: 16 * r + 16,
                                                           c0:c0 + w],
                            in_=t16[:, :w])
            # stage xs0 shard into internal dram, AG to full table
            nc.sync.dma_start(out=xs0sh.ap(), in_=xs0p.ap())
            allgather(g, xs0sh, XS[0])
            # =========== DOWN LEVEL 0 ===========
            conv_pass(XS[0], [("xs", BF16, u0u0, xsout[1], bbc[1], u0c)],
                      W_next=Wt[1])
            allgather(g, xsout[1], XS[1])
            conv_pass(XS[1], [("xs", BF16, u0q0, xsout[2]),
                              ("xs", BF16, u0c, h2save)])
            allgather(g, xsout[2], XS[2])
            score0 = nv(g, "score0")
            conv_pass(XS[2], [("xs", BF16, r0c, h3tmp)], W_next=Wt[2],
                      score_to=score0, colfac=r0c, pcol=p0c)
            nc.sync.dma_start(out=score_sh_d[0].ap(), in_=score0[:])
            allgather(g, score_sh_d[0], score_fl_d[0])
            sc0f = load_full_from_ag(g, score_fl_d[0], "sc0f")
            sel0f = sel_from(g, sc0f, rm_fl, "sel0f")
            nsc0f = nv(g, "nsc0f", [P, g.BT])
            nc.vector.tensor_scalar(out=nsc0f[:], in0=sc0f[:], scalar1=-1.0,
                                    scalar2=None, op0=OP.mult)
            neg0f = sel_from(g, nsc0f, rm_fl, "neg0f")
            k0 = math.ceil(cfg.ratio * cfg.N)
            thr0 = bisect_topk(g, sel0f, k0, neg0f, "thr0")
            kept0f = nv(g, "kept0f", [P, g.BT])
            nc.vector.tensor_scalar(out=kept0f[:], in0=sel0f[:], scalar1=thr0[:],
                                    scalar2=None, op0=OP.is_ge)
            sel0s = sel_from(g, score0, rm_sh, "sel0s")
            kept0s = nv(g, "kept0s")
            nc.vector.tensor_scalar(out=kept0s[:], in0=sel0s[:], scalar1=thr0[:],
                                    scalar2=None, op0=OP.is_ge)
            tanh0 = nv(g, "tanh0")
            nc.scalar.activation(out=tanh0[:], in_=score0[:], func=AF.Tanh)
            # Z pass A: deg1 raw (rs direction: gather kept0 at col, segsum at row)
            zbuild(g, [kept0f])
            S1 = nv(g, "S1")
            emit_dir_pass(g, "rs", None, "z", z_block_fn(g, [], [(S1, 0)], 1),
                          zcols=1)
            deg1 = vmul(kept0s, S1, "deg1")
            m1 = nv(g, "m1")
            nc.vector.tensor_scalar(out=m1[:], in0=deg1[:], scalar1=0.0, scalar2=None,
                                    op0=OP.is_gt)
            dsafe = nv(g, "dsafe")
            nc.vector.tensor_scalar(out=dsafe[:], in0=deg1[:], scalar1=1e-30,
                                    scalar2=None, op0=OP.max)
            u1 = nv(g, "u1")
            nc.vector.reciprocal(out=u1[:], in_=dsafe[:])
            nc.scalar.activation(out=u1[:], in_=u1[:], func=AF.Sqrt)
            nc.vector.tensor_tensor(out=u1[:], in0=u1[:], in1=m1[:], op=OP.mult)
            w1 = vmul(aggr0c, kept0s, "w1")
            rdeg1 = nv(g, "rdeg1")
            nc.vector.reciprocal(out=rdeg1[:], in_=dsafe[:])
            q1 = vmul(w1, rdeg1, "q1")
            nc.vector.tensor_tensor(out=q1[:], in0=q1[:], in1=m1[:], op=OP.mult)
            nc.sync.dma_start(out=q1_sh_d.ap(), in_=q1[:])
            allgather(g, q1_sh_d, q1_fl_d)
            q1f = load_full_from_ag(g, q1_fl_d, "q1f")
            # Z pass B: A1 (col)  (cs direction, gather at row)
            zbuild(g, [q1f])
            A1 = nv(g, "A1")
            emit_dir_pass(g, "cs", None, "z",
                          z_block_fn(g, [], [(A1, 0)], 1), zcols=1)
            aggr1 = vmul(kept0s, A1, "aggr1")
            nc.vector.tensor_scalar(out=aggr1[:], in0=aggr1[:], scalar1=1e-12,
                                    scalar2=None, op0=OP.add)
            raggr1 = nv(g, "raggr1")
            nc.vector.reciprocal(out=raggr1[:], in_=aggr1[:])
            r1 = vmul(kept0s, raggr1, "r1")
            cv3 = vmul(tanh0, u1, "cv3")
            rescale_pass(h3tmp, xsout[3], cv3, bbc=bbc[2], bcol=u1)
            allgather(g, xsout[3], XS[3])

            # =========== DOWN LEVEL 1 ===========
            u1u1 = vmul(u1, u1, "u1u1")
            u1q1 = vmul(u1, q1, "u1q1")
            conv_pass(XS[3], [("xs", BF16, u1u1, xsout[4], bbc[3], u1)],
                      W_next=Wt[3])
            allgather(g, xsout[4], XS[4])
            conv_pass(XS[4], [("xs", BF16, u1q1, xsout[5]),
                              ("xs", BF16, u1, h5save)])
            allgather(g, xsout[5], XS[5])
            score1 = nv(g, "score1")
            conv_pass(XS[5], [("xs", BF16, r1, h6tmp)], W_next=Wt[4],
                      score_to=score1, colfac=r1, pcol=p1c)
            nc.sync.dma_start(out=score_sh_d[1].ap(), in_=score1[:])
            allgather(g, score_sh_d[1], score_fl_d[1])
            sc1f = load_full_from_ag(g, score_fl_d[1], "sc1f")
            sel1f = sel_from(g, sc1f, kept0f, "sel1f")
            nsc1f = nv(g, "nsc1f", [P, g.BT])
            nc.vector.tensor_scalar(out=nsc1f[:], in0=sc1f[:], scalar1=-1.0,
                                    scalar2=None, op0=OP.mult)
            neg1f = sel_from(g, nsc1f, kept0f, "neg1f")
            k1 = math.ceil(cfg.ratio * k0)
            thr1 = bisect_topk(g, sel1f, k1, neg1f, "thr1")
            kept1f = nv(g, "kept1f", [P, g.BT])
            nc.vector.tensor_scalar(out=kept1f[:], in0=sel1f[:], scalar1=thr1[:],
                                    scalar2=None, op0=OP.is_ge)
            sel1s = sel_from(g, score1, kept0s, "sel1s")
            kept1s = nv(g, "kept1s")
            nc.vector.tensor_scalar(out=kept1s[:], in0=sel1s[:], scalar1=thr1[:],
                                    scalar2=None, op0=OP.is_ge)
            tanh1 = nv(g, "tanh1")
            nc.scalar.activation(out=tanh1[:], in_=score1[:], func=AF.Tanh)
            # Z pass C: deg2 raw
            zbuild(g, [kept1f])
            S2 = nv(g, "S2")
            emit_dir_pass(g, "rs", None, "z", z_block_fn(g, [], [(S2, 0)], 1),
                          zcols=1)
            deg2 = vmul(kept1s, S2, "deg2")
            m2 = nv(g, "m2")
            nc.vector.tensor_scalar(out=m2[:], in0=deg2[:], scalar1=0.0, scalar2=None,
                                    op0=OP.is_gt)
            d2safe = nv(g, "d2safe")
            nc.vector.tensor_scalar(out=d2safe[:], in0=deg2[:], scalar1=1e-30,
                                    scalar2=None, op0=OP.max)
            u2 = nv(g, "u2")
            nc.vector.reciprocal(out=u2[:], in_=d2safe[:])
            nc.scalar.activation(out=u2[:], in_=u2[:], func=AF.Sqrt)
            nc.vector.tensor_tensor(out=u2[:], in0=u2[:], in1=m2[:], op=OP.mult)
            cv6 = vmul(tanh1, u2, "cv6")
            rescale_pass(h6tmp, xsout[6], cv6, bbc=bbc[4], bcol=u2)
            allgather(g, xsout[6], XS[6])

            # =========== BOTTOM ===========
            u2u2 = vmul(u2, u2, "u2u2")
            u2r1 = vmul(u2, r1, "u2r1")
            conv_pass(XS[6], [("xs", BF16, u2u2, xsout[7], bbc[5], u2)],
                      W_next=Wt[5])
            allgather(g, xsout[7], XS[7])
            conv_pass(XS[7], [("xs", BF16, u2r1, xsout[8])])
            allgather(g, xsout[8], XS[8])

            # =========== UP LEVEL (emask1): wec-up + 2 convs ===========
            q1u1 = vmul(q1, u1, "q1u1")
            conv_pass(XS[8], [("xs", BF16, q1u1, xsout[9], bbc[6], u1)],
                      W_next=Wt[6], dirn="rs")
            allgather(g, xsout[9], XS[9])
            conv_pass(XS[9], [("xs", BF16, u1u1, xsout[10], bbc[7], u1)],
                      W_next=Wt[7])
            allgather(g, xsout[10], XS[10])
            conv_pass(XS[10], [("add", BF16, u1, xsout[11], h5save, r0c)])
            allgather(g, xsout[11], XS[11])

            # =========== UP LEVEL (emask0) ===========
            q0u0 = vmul(q0c, u0c, "q0u0")
            conv_pass(XS[11], [("xs", BF16, q0u0, xsout[12], bbc[8], u0c)],
                      W_next=Wt[8], dirn="rs")
            allgather(g, xsout[12], XS[12])
            conv_pass(XS[12], [("xs", BF16, u0u0, xsout[13], bbc[9], u0c)],
                      W_next=Wt[9])
            allgather(g, xsout[13], XS[13])
            conv_pass(XS[13], [("add", F32, u0c, out_p, h2save, None)])

        emit_schedule()

        for c in reversed(ctxs):
            c.__exit__(None, None, None)

    nc.compile()
    return nc


def make_inmaps(cfg, meta):
    cs, rs = meta["cs"], meta["rs"]
    bpc = cfg.blocks_per_core
    sv = meta["shardvec"]
    u0s, q0s, r0s, ag0s = (sv(meta[k]) for k in ("u0", "q0", "r0", "aggr0"))
    rms = sv(meta["realmask"])
    rmf = meta["fullvec"](meta["realmask"])
    iotas = np.tile(np.arange(P, dtype=np.float32)[None, :], (P, 1)).astype(bfloat16)
    ident = np.eye(P, dtype=np.float32)
    ones = np.ones((P, 2), np.float32)
    pcol = np.stack([meta["p0"], meta["p1"]], 1).astype(bfloat16)

    in_maps = []
    for c in range(NCORES):
        statcv = np.zeros((P, bpc, 6), np.float32)
        statcv[:, :, 0] = u0s[c]
        statcv[:, :, 1] = q0s[c]
        statcv[:, :, 2] = r0s[c]
        statcv[:, :, 3] = ag0s[c]
        statcv[:, :, 4] = rms[c]
        in_maps.append({
            "xs0": None,
            "idx16_cs": cs["idx16"][c], "dstl_cs": cs["dstl"][c],
            "idx16_rs": rs["idx16"][c], "dstl_rs": rs["dstl"][c],
            "statc": statcv,
            "realmask_full": rmf,
            "iotasmall": iotas, "ident": ident, "ones": ones, "pcol": pcol,
            "wall": None, "ballbc": None,
        })
    return in_maps


def fill_weights(cfg, meta, in_maps, Wd, bd, Wu, bu, Wb, bb):
    Ws = [Wd[0, 0], Wd[0, 1], Wd[1, 0], Wd[1, 1], Wb[0], Wb[1],
          Wu[0, 0], Wu[0, 1], Wu[1, 0], Wu[1, 1]]
    bs = [bd[0, 0], bd[0, 1], bd[1, 0], bd[1, 1], bb[0], bb[1],
          bu[0, 0], bu[0, 1], bu[1, 0], bu[1, 1]]
    wall = np.concatenate([w.astype(np.float32) for w in Ws], 0).astype(bfloat16)
    ballbc = np.tile(np.stack([b.astype(np.float32) for b in bs], 0)[None],
                     (P, 1, 1)).reshape(P, -1).astype(bfloat16)
    # XS0 table: (x @ W0 + b0) * u0, padded + sharded
    N, Np, npc = cfg.N, cfg.N_pad, cfg.nodes_per_core
    xs0 = np.zeros((Np, cfg.D), np.float32)
    xs0[:N] = (meta["x"] @ Ws[0].astype(np.float32) + bs[0].astype(np.float32)) \
        * meta["u0"][:N, None]
    xs0b = xs0.astype(bfloat16)
    for c, m in enumerate(in_maps):
        m["wall"] = wall
        m["ballbc"] = ballbc
        m["xs0"] = xs0b[c * npc:(c + 1) * npc]


def run_gnn(cfg, inputs, nc_cache={}, full_pad=False, trace=False):
    """Full pipeline: preprocess, build (cached by cfg), run, assemble."""
    x = np.asarray(inputs["x"], np.float32)
    ei = np.asarray(inputs["edge_index"])
    pvec = np.asarray(inputs["pvec"], np.float32)
    meta = preprocess(cfg, x, ei, pvec)
    key = (cfg.N, cfg.E, ei.tobytes()[:64])  # program depends on edge stats
    if key not in nc_cache:
        nc_cache.clear()
        nc_cache[key] = (build_kernel(cfg, meta), None)
    nc, _ = nc_cache[key]
    in_maps = make_inmaps(cfg, meta)
    fill_weights(cfg, meta, in_maps,
                 *(np.asarray(inputs[k], np.float32)
                   for k in ("Wd", "bd", "Wu", "bu", "Wb", "bb")))
    res = run_bass_kernel_spmd(nc, in_maps, list(range(NCORES)), trace=trace)
    out = np.concatenate([res.results[c]["out"] for c in range(NCORES)], 0)
    return (out if full_pad else out[: cfg.N]), res


_CFG = Cfg()


def kernel(**inputs):
    out, _ = run_gnn(_CFG, inputs)
    return out.astype(np.float32)
